# revision 2
# baseline (speedup 1.0000x reference)
"""Causal multi-head attention on 8 Trainium2 NeuronCores.

Problem: x[2,2048,1024] @ W_Q/K/V[1024,1024] -> 16-head causal attention
(d_head=64) -> @ W_O[1024,1024].

Sharding: tensor-parallel over heads. Core i owns heads 2i, 2i+1 — i.e.
columns [128i:128i+128) of W_Q/W_K/W_V and rows [128i:128i+128) of W_O.
Each core computes its partial output [1024, 4096] (transposed layout);
the host sums the 8 partials and un-transposes (the "all-reduce").

Device kernel (per core, all matmuls in float32r = full-rate fp32):
  1. Projections from xT [1024, 4096] (host pre-transposes x):
     QT/KT [128, 4096] = W.T @ xT; V in natural [token, dim] layout via
     PE transpose, with a ones-column appended per head (65-wide blocks)
     so the PV matmul also produces the softmax denominator for free.
  2. Flash-style causal attention with scores in [k, q] orientation:
     scoresT = KT.T-slice @ QT-slice, exp on ScalarE (no max-subtraction:
     scores ~ N(0,1), exp is safe in fp32), causal mask applied
     multiplicatively on the 4 diagonal chunk variants only, PV matmul
     accumulates [65, 512] (64 dims + denominator row) in PSUM.
  3. Normalize by the denominator row (reciprocal + partition broadcast),
     then outT_partial = W_O-slice.T @ attnT.
"""

import numpy as np

import concourse.bass as bass
import concourse.tile as tile
from concourse import bacc, mybir
from concourse.bass_utils import run_bass_kernel_spmd
from concourse.masks import make_identity

F32 = mybir.dt.float32
F32R = mybir.dt.float32r

N_CORES = 8
P = 128
D = 1024          # d_model
B = 2             # batch
S = 2048          # seq len
T = B * S         # total tokens = 4096
TT = 512          # token tile (free dim of matmuls)
NT = T // TT      # 8 token tiles
KD = D // P       # 8 contraction chunks for projections
JB = S // TT      # 4 q-tiles per batch
CB = S // P       # 16 k-chunks per batch
NCH = T // P      # 32 k-chunks total
H_LOC = 2         # heads per core
DH = 64           # head dim


def _body(tc):
    nc = tc.nc
    xT = nc.dram_tensor("xT", [D, T], F32R, kind="ExternalInput").ap()
    wq = nc.dram_tensor("wq", [D, P], F32R, kind="ExternalInput").ap()
    wk = nc.dram_tensor("wk", [D, P], F32R, kind="ExternalInput").ap()
    wv = nc.dram_tensor("wv", [D, P], F32R, kind="ExternalInput").ap()
    wo = nc.dram_tensor("wo", [P, D], F32R, kind="ExternalInput").ap()
    outT = nc.dram_tensor("outT", [D, T], F32, kind="ExternalOutput").ap()

    import contextlib
    with contextlib.ExitStack() as ctx:
        const = ctx.enter_context(tc.tile_pool(name="const", bufs=1))
        wpool = ctx.enter_context(tc.tile_pool(name="wpool", bufs=1))
        xpool = ctx.enter_context(tc.tile_pool(name="xpool", bufs=2))
        persist = ctx.enter_context(tc.tile_pool(name="persist", bufs=1))
        probs_p = ctx.enter_context(tc.tile_pool(name="probs", bufs=10))
        stage = ctx.enter_context(tc.tile_pool(name="stage", bufs=3))
        psum = ctx.enter_context(tc.tile_pool(name="psum", bufs=4, space="PSUM"))

        # --- constants -----------------------------------------------------
        identity = const.tile([P, P], F32)
        make_identity(nc, identity)

        # mask_band[k, q] = 1.0 if q >= k else 0.0 (lower-left triangular 0s)
        mask_band = const.tile([P, P], F32)
        nc.any.memset(mask_band[:], 1.0)
        nc.gpsimd.affine_select(
            out=mask_band[:],
            in_=mask_band[:],
            compare_op=mybir.AluOpType.is_ge,
            fill=0.0,
            base=0,
            pattern=[[1, P]],
            channel_multiplier=-1,
        )

        # --- weights -------------------------------------------------------
        wq_sb = wpool.tile([P, KD, P], F32R)
        nc.sync.dma_start(wq_sb[:], wq.rearrange("(o p) m -> p o m", p=P))
        wk_sb = wpool.tile([P, KD, P], F32R)
        nc.sync.dma_start(wk_sb[:], wk.rearrange("(o p) m -> p o m", p=P))
        wv_sb = wpool.tile([P, KD, P], F32R)
        nc.sync.dma_start(wv_sb[:], wv.rearrange("(o p) m -> p o m", p=P))
        wo_sb = wpool.tile([P, D], F32R)
        nc.sync.dma_start(wo_sb[:], wo)

        # --- persistent activations ---------------------------------------
        qT = persist.tile([P, T], F32R)       # [2h x 64d, tokens]
        kT = persist.tile([P, T], F32R)
        vn = persist.tile([P, NCH, 130], F32R)  # [token, chunk, d0|1|d1|1]
        attnT = persist.tile([P, T], F32R)
        for col in (DH, 2 * DH + 1):
            nc.scalar.activation(vn[:, :, col], vn[:, :, col],
                                 mybir.ActivationFunctionType.Identity,
                                 bias=1.0, scale=0.0)

        xT_r = xT.rearrange("(o p) n -> p o n", p=P)
        outT_r = outT.rearrange("(o p) n -> p o n", p=P)

        # --- phase 1: projections -----------------------------------------
        for t in range(NT):
            xt = xpool.tile([P, KD, TT], F32R)
            for c in range(KD):
                nc.sync.dma_start(xt[:, c, :], xT_r[:, c, bass.ts(t, TT)])
            for wsb, dstT in ((wq_sb, qT), (wk_sb, kT)):
                ps = psum.tile([P, TT], F32, tag="a")
                for c in range(KD):
                    nc.tensor.matmul(ps[:], wsb[:, c, :], xt[:, c, :],
                                     start=(c == 0), stop=(c == KD - 1))
                nc.vector.tensor_copy(dstT[:, bass.ts(t, TT)], ps[:])
            # V: project, then PE-transpose into natural [token, dim] layout
            ps = psum.tile([P, TT], F32, tag="a")
            for c in range(KD):
                nc.tensor.matmul(ps[:], wv_sb[:, c, :], xt[:, c, :],
                                 start=(c == 0), stop=(c == KD - 1))
            vt = stage.tile([P, TT], F32, tag="vt")
            nc.vector.tensor_copy(vt[:], ps[:])
            for s_ in range(4):
                pt = psum.tile([P, P], F32, tag="b")
                nc.tensor.transpose(pt[:], vt[:, bass.ts(s_, P)], identity)
                ch = t * 4 + s_
                nc.vector.tensor_copy(vn[:, ch, 0:DH], pt[:, 0:DH])
                nc.vector.tensor_copy(vn[:, ch, DH + 1:2 * DH + 1],
                                      pt[:, DH:2 * DH])

        # --- phase 2: causal attention ------------------------------------
        # Dual-j: the same-index q-tiles of batch 0 and batch 1 are
        # processed together (same causal shape), doubling the independent
        # matmul streams in flight. Lag-1 software pipeline: the PV matmul
        # for chunk cb-1 is emitted after the scores matmul for chunk cb so
        # the PE never waits on ScalarE's exp. Diagonal chunk r: exp/PV only
        # the live columns [128r:], triangular mask on the 128-wide band.
        for jj in range(JB):
            ncb = 4 * (jj + 1)
            js = (jj, jj + JB)
            pvs = {}
            for jx in js:
                for h in range(H_LOC):
                    pvs[(jx, h)] = psum.tile([DH + 1, TT], F32, tag="a",
                                             name=f"pv_{jx}_{h}")

            def pv_step(jx, cb, prs, jj=jj, ncb=ncb, pvs=pvs):
                b = jx // JB
                c = CB * b + cb
                r = cb - 4 * jj
                lo = P * r if r > 0 else 0
                for h in range(H_LOC):
                    nc.tensor.matmul(pvs[(jx, h)][:, lo:],
                                     vn[:, c, bass.ds((DH + 1) * h, DH + 1)],
                                     prs[h][:, lo:],
                                     start=(cb == 0), stop=(cb == ncb - 1))

            pending = {}
            for cb in range(ncb):
                r = cb - 4 * jj
                lo = P * r if r > 0 else 0
                for jx in js:
                    b = jx // JB
                    c = CB * b + cb
                    csl = bass.ts(c, P)
                    jsl = bass.ts(jx, TT)
                    prs = []
                    for h in range(H_LOC):
                        hp = slice(DH * h, DH * h + DH)
                        sps = psum.tile([P, TT], F32, tag="b",
                                        name=f"sps_{jx}_{cb}_{h}")
                        nc.tensor.matmul(sps[:, lo:], kT[hp, csl],
                                         qT[hp, jsl][:, lo:],
                                         start=True, stop=True)
                        pr = probs_p.tile([P, TT], F32R, tag="pr",
                                          name=f"pr_{jx}_{cb}_{h}")
                        nc.scalar.activation(pr[:, lo:], sps[:, lo:],
                                             mybir.ActivationFunctionType.Exp,
                                             scale=0.125)
                        if r >= 0:
                            nc.vector.tensor_mul(pr[:, bass.ts(r, P)],
                                                 pr[:, bass.ts(r, P)],
                                                 mask_band[:])
                        prs.append(pr)
                    if jx in pending:
                        pv_step(jx, cb - 1, pending[jx])
                    pending[jx] = prs
            for jx in js:
                pv_step(jx, ncb - 1, pending[jx])

            for jx in js:
                jsl = bass.ts(jx, TT)
                for h in range(H_LOC):
                    hp = slice(DH * h, DH * h + DH)
                    rc = stage.tile([1, TT], F32, tag="rc",
                                    name=f"rc_{jx}_{h}")
                    nc.vector.reciprocal(rc[:], pvs[(jx, h)][DH:DH + 1, :])
                    rb = stage.tile([DH, TT], F32, tag="rb",
                                    name=f"rb_{jx}_{h}")
                    nc.gpsimd.partition_broadcast(rb[:], rc[:])
                    nc.vector.tensor_mul(attnT[hp, jsl],
                                         pvs[(jx, h)][0:DH, :], rb[:])

        # --- phase 3: output projection (partial) -------------------------
        for j in range(NT):
            for f in range(KD):
                wps = psum.tile([P, TT], F32, tag="b", name=f"wps_{j}_{f}")
                nc.tensor.matmul(wps[:], wo_sb[:, bass.ts(f, P)],
                                 attnT[:, bass.ts(j, TT)],
                                 start=True, stop=True)
                ob = stage.tile([P, TT], F32, tag="ob", name=f"ob_{j}_{f}")
                nc.vector.tensor_copy(ob[:], wps[:])
                nc.sync.dma_start(outT_r[:, f, bass.ts(j, TT)], ob[:])


_NC_CACHE = None


def _get_nc():
    global _NC_CACHE
    if _NC_CACHE is None:
        nc = bacc.Bacc("TRN2", target_bir_lowering=False, debug=False,
                       num_devices=N_CORES)
        with tile.TileContext(nc) as tc:
            _body(tc)
        nc.compile()
        _NC_CACHE = nc
    return _NC_CACHE


def _in_maps(x, W_Q, W_K, W_V, W_O):
    xT = np.ascontiguousarray(
        np.asarray(x, dtype=np.float32).reshape(T, D).T)
    W_Q = np.asarray(W_Q, dtype=np.float32)
    W_K = np.asarray(W_K, dtype=np.float32)
    W_V = np.asarray(W_V, dtype=np.float32)
    W_O = np.asarray(W_O, dtype=np.float32)
    maps = []
    for i in range(N_CORES):
        sl = slice(P * i, P * i + P)
        maps.append({
            "xT": xT,
            "wq": np.ascontiguousarray(W_Q[:, sl]),
            "wk": np.ascontiguousarray(W_K[:, sl]),
            "wv": np.ascontiguousarray(W_V[:, sl]),
            "wo": np.ascontiguousarray(W_O[sl, :]),
        })
    return maps


def _gather(results):
    acc = np.zeros([D, T], np.float64)
    for r in results:
        acc += r["outT"]
    return np.ascontiguousarray(
        acc.T.astype(np.float32)).reshape(B, S, D)


def kernel(x, W_Q, W_K, W_V, W_O):
    nc = _get_nc()
    res = run_bass_kernel_spmd(nc, _in_maps(x, W_Q, W_K, W_V, W_O),
                               core_ids=list(range(N_CORES)))
    return _gather(res.results)


LAST_RESULT = None


def kernel_profiled(x, W_Q, W_K, W_V, W_O):
    """Like kernel() but with NTFF tracing; returns (output, exec_time_ns)."""
    import os
    global LAST_RESULT
    nc = _get_nc()
    res = run_bass_kernel_spmd(nc, _in_maps(x, W_Q, W_K, W_V, W_O),
                               core_ids=list(range(N_CORES)), trace=True,
                               tmpdir=os.environ.get("BASS_TRACE_DIR"))
    LAST_RESULT = res
    return _gather(res.results), res.exec_time_ns



# revision 5
# speedup vs baseline: 2.1828x; 2.1828x over previous
"""Causal multi-head attention on 8 Trainium2 NeuronCores — v2.

Problem: x[2,2048,1024] @ W_Q/K/V[1024,1024] -> 16-head causal attention
(d_head=64) -> @ W_O[1024,1024].

Sharding: DP(batch=2) x TP(head-groups=4). Core i owns batch i//4 and
heads 4k..4k+3 where k = i%4 (columns [256k:256k+256) of W_Q/K/V, rows
[256k:256k+256) of W_O). Each core emits a partial [1024, 2048] output
for its batch; host sums groups of 4 and transposes.

All matmul inputs bf16 (f32 PSUM accumulate). Design notes:
  - Attention runs in two "waves" (head pairs), so PSUM fits: scores
    double-buffered [128,2,512] (4 banks) + PV accum [65,2,512]
    (2 banks) + utility [128,512] x2 (2 banks) = 8 banks.
  - exp batched over the head pair: one ACTIVATE per (jj, chunk) covers
    [128, 2, width] -> halves ScalarE fixed overhead.
  - V projected directly in [token, dim] orientation (x-chunk stationary,
    W_V moving) — no PE transposes.
  - Projections for tile t+1 are emitted interleaved into wave-A jj=t's
    chunk loop; W_O matmuls for tile jx interleave into wave-B jj=jx+1.
    Keeps the PE stream dense so the HAM clock gate stays at 2.4 GHz.
  - W_O results bounce PSUM->SBUF as bf16, stored + host-summed as bf16
    partials (halves store DMA).
  - Softmax denominator via ones-column in vn ([65]-wide PV stationary);
    normalize = PSUM->SBUF copy, reciprocal_approx_fast on the denominator
    row, gpsimd partition broadcast, DVE multiply — off the PE path.
"""

import numpy as np

import concourse.bass as bass
import concourse.tile as tile
from concourse import bacc, mybir
from concourse.bass_utils import run_bass_kernel_spmd

F32 = mybir.dt.float32
BF16 = mybir.dt.bfloat16

N_CORES = 8
P = 128
D = 1024          # d_model
B = 2             # batch
S = 2048          # seq len (= tokens per core)
TT = 512          # token tile
NT = S // TT      # 4 token tiles
KD = D // P       # 8 contraction chunks for projections
CB = S // P       # 16 k-chunks
HL = 4            # heads per core
DL = 256          # dims per core (2 ptiles of 128)
DH = 64           # head dim
EXP = mybir.ActivationFunctionType.Exp


def _body(tc):
    nc = tc.nc
    xT = nc.dram_tensor("xT", [D, S], BF16, kind="ExternalInput").ap()
    wq = nc.dram_tensor("wq", [D, DL], BF16, kind="ExternalInput").ap()
    wk = nc.dram_tensor("wk", [D, DL], BF16, kind="ExternalInput").ap()
    wv = nc.dram_tensor("wv", [D, DL], BF16, kind="ExternalInput").ap()
    wo = nc.dram_tensor("wo", [DL, D], BF16, kind="ExternalInput").ap()
    outT = nc.dram_tensor("outT", [D, S], BF16, kind="ExternalOutput").ap()

    import contextlib
    with contextlib.ExitStack() as ctx:
        const = ctx.enter_context(tc.tile_pool(name="const", bufs=1))
        wpool = ctx.enter_context(tc.tile_pool(name="wpool", bufs=1))
        xpool = ctx.enter_context(tc.tile_pool(name="xpool", bufs=2))
        persist = ctx.enter_context(tc.tile_pool(name="persist", bufs=1))
        probs_p = ctx.enter_context(tc.tile_pool(name="probs", bufs=4))
        stage = ctx.enter_context(tc.tile_pool(name="stage", bufs=2))
        psum = ctx.enter_context(tc.tile_pool(name="psum", bufs=1, space="PSUM"))

        # --- constants -----------------------------------------------------
        # mask_band[k, q] = 1.0 if q >= k else 0.0
        mask_band = const.tile([P, P], BF16)
        nc.any.memset(mask_band[:], 1.0)
        nc.gpsimd.affine_select(
            out=mask_band[:],
            in_=mask_band[:],
            compare_op=mybir.AluOpType.is_ge,
            fill=0.0,
            base=0,
            pattern=[[1, P]],
            channel_multiplier=-1,
        )
        # preload the exp table set while projections run
        scr = const.tile([1, 1], F32)
        nc.any.memset(scr[:], 0.0)
        nc.scalar.activation(scr[:], scr[:], EXP)

        # --- weights -------------------------------------------------------
        wq_sb = wpool.tile([P, KD, DL], BF16)
        nc.sync.dma_start(wq_sb[:], wq.rearrange("(o p) m -> p o m", p=P))
        wk_sb = wpool.tile([P, KD, DL], BF16)
        nc.sync.dma_start(wk_sb[:], wk.rearrange("(o p) m -> p o m", p=P))
        wv_sb = wpool.tile([P, KD, DL], BF16)
        nc.sync.dma_start(wv_sb[:], wv.rearrange("(o p) m -> p o m", p=P))
        wo_sb = wpool.tile([P, 2, D], BF16)
        nc.sync.dma_start(wo_sb[:], wo.rearrange("(c p) m -> p c m", p=P))

        # --- persistent activations ---------------------------------------
        qT = persist.tile([P, 2, S], BF16)    # [dim%128, ptile, token]
        kT = persist.tile([P, 2, S], BF16)
        attnT = persist.tile([P, 2, S], BF16)
        vn = persist.tile([P, CB, HL, DH + 1], BF16)  # [tok, chunk, head, d|1]
        nc.any.memset(vn[:, :, :, DH], 1.0)
        pvsb = persist.tile([DH + 1, NT, 2, TT], F32)  # [d|1, jj, hh, q]

        xT_r = xT.rearrange("(o p) n -> p o n", p=P)
        outT_r = outT.rearrange("(o p) n -> p o n", p=P)

        # --- projection work for one token tile, as emission groups -------
        def proj_tile_groups(t):
            tsl = bass.ts(t, TT)
            xt = xpool.tile([P, KD, TT], BF16, name=f"xt_{t}")

            def load():
                for c in range(KD):
                    nc.sync.dma_start(xt[:, c, :], xT_r[:, c, tsl])
            groups = [load]

            def qk_chain(wsb, dst, pt):
                def run():
                    ps = psum.tile([P, TT], F32, tag="u", bufs=2,
                                   name=f"ps_{t}_{pt}")
                    for c in range(KD):
                        nc.tensor.matmul(ps[:], wsb[:, c, bass.ts(pt, P)],
                                         xt[:, c, :],
                                         start=(c == 0), stop=(c == KD - 1))
                    nc.vector.tensor_copy(dst[:, pt, tsl], ps[:])
                return run

            for wsb, dst in ((wq_sb, qT), (wk_sb, kT)):
                for pt in range(2):
                    groups.append(qk_chain(wsb, dst, pt))

            # V directly in [token, dim] orientation: x chunk stationary.
            def v_chunk(s_):
                def run():
                    ch = t * 4 + s_
                    ps = psum.tile([P, DL], F32, tag="u", bufs=2,
                                   name=f"psv_{t}_{s_}")
                    for c in range(KD):
                        nc.tensor.matmul(ps[:], xt[:, c, bass.ts(s_, P)],
                                         wv_sb[:, c, :],
                                         start=(c == 0), stop=(c == KD - 1))
                    # [128 tok, 4*64 dims] -> vn[:, ch, h, 0:64]
                    nc.vector.tensor_copy(
                        vn[:, ch, :, 0:DH],
                        ps.rearrange("p (h d) -> p h d", h=HL))
                return run

            for s_ in range(4):
                groups.append(v_chunk(s_))
            return groups

        # --- W_O for one token tile, as one emission group ----------------
        def wo_group(jx):
            jsl = bass.ts(jx, TT)

            def run():
                for f in range(KD):
                    wu = psum.tile([P, TT], F32, tag="u", bufs=2,
                                   name=f"wu_{jx}_{f}")
                    nc.tensor.matmul(wu[:], wo_sb[:, 0, bass.ts(f, P)],
                                     attnT[:, 0, jsl], start=True, stop=False)
                    nc.tensor.matmul(wu[:], wo_sb[:, 1, bass.ts(f, P)],
                                     attnT[:, 1, jsl], start=False, stop=True)
                    ob = stage.tile([P, TT], BF16, tag="ob", bufs=3,
                                    name=f"ob_{jx}_{f}")
                    nc.vector.tensor_copy(ob[:], wu[:])
                    nc.sync.dma_start(outT_r[:, f, jsl], ob[:])
            return run

        # --- attention wave: one head pair (ptile), all q-tiles -----------
        def norm_jj(pt, jj):
            jsl = bass.ts(jj, TT)
            for hh in range(2):
                # reciprocal_approx_fast misreads inputs at base partition
                # 64 (custom-DVE uop quirk) — bounce the row to partition 0.
                dcp = stage.tile([1, TT], F32, tag="dcp",
                                 name=f"dcp_{pt}_{jj}_{hh}")
                nc.vector.tensor_copy(dcp[:], pvsb[DH:DH + 1, jj, hh, :])
                rc = stage.tile([1, TT], F32, tag="rc",
                                name=f"rc_{pt}_{jj}_{hh}")
                nc.vector.reciprocal_approx_fast(out=rc[:], in_=dcp[:])
                rb = stage.tile([DH, TT], F32, tag="rb",
                                name=f"rb_{pt}_{jj}_{hh}")
                nc.gpsimd.partition_broadcast(rb[:], rc[:])
                nc.vector.tensor_mul(
                    attnT[DH * hh:DH * hh + DH, pt, jsl],
                    pvsb[0:DH, jj, hh, :], rb[:])

        def wave(pt, extra_per_jj):
            """extra_per_jj[jj]: emission groups interleaved into jj's chunk
            loop; all are emitted before jj's normalize (so before jj+1)."""
            for jj in range(NT):
                extra = list(extra_per_jj[jj])
                ncb = 4 * (jj + 1)
                jsl = bass.ts(jj, TT)
                pv = psum.tile([DH + 1, 2, TT], F32, tag="pv", bufs=1,
                               name=f"pv_{pt}_{jj}")

                def pv_step(cb, pr, jj=jj, ncb=ncb, pv=pv):
                    r = cb - 4 * jj
                    lo = P * r if r > 0 else 0
                    for hh in range(2):
                        nc.tensor.matmul(pv[:, hh, lo:],
                                         vn[:, cb, 2 * pt + hh, :],
                                         pr[:, hh, lo:],
                                         start=(cb == 0), stop=(cb == ncb - 1))

                pending = None
                for cb in range(ncb):
                    r = cb - 4 * jj
                    lo = P * r if r > 0 else 0
                    csl = bass.ts(cb, P)
                    sps = psum.tile([P, 2, TT], F32, tag="s", bufs=2,
                                    name=f"sps_{pt}_{jj}_{cb}")
                    for hh in range(2):
                        hp = slice(DH * hh, DH * hh + DH)
                        nc.tensor.matmul(sps[:, hh, lo:], kT[hp, pt, csl],
                                         qT[hp, pt, jsl][:, lo:],
                                         start=True, stop=True)
                    pr = probs_p.tile([P, 2, TT], BF16, tag="pr",
                                      name=f"pr_{pt}_{jj}_{cb}")
                    nc.scalar.activation(pr[:, :, lo:], sps[:, :, lo:],
                                         EXP, scale=0.125)
                    if r >= 0:
                        for hh in range(2):
                            nc.vector.tensor_mul(pr[:, hh, bass.ts(r, P)],
                                                 pr[:, hh, bass.ts(r, P)],
                                                 mask_band[:])
                    if pending is not None:
                        pv_step(cb - 1, pending)
                    pending = pr
                    # spread the extra groups across remaining chunk slots
                    k = -(-len(extra) // (ncb - cb)) if extra else 0
                    for _ in range(k):
                        extra.pop(0)()
                pv_step(ncb - 1, pending)
                while extra:
                    extra.pop(0)()

                # drain PV to SBUF (frees the 2 PSUM banks), normalize
                nc.vector.tensor_copy(pvsb[:, jj, :, :], pv[:])
                norm_jj(pt, jj)

        # --- schedule ------------------------------------------------------
        for g in proj_tile_groups(0):
            g()
        wave(0, [proj_tile_groups(t) for t in range(1, NT)] + [[]])
        wave(1, [[]] + [[wo_group(jx)] for jx in range(NT - 1)])
        wo_group(NT - 1)()


_NC_CACHE = None


def _get_nc():
    global _NC_CACHE
    if _NC_CACHE is None:
        nc = bacc.Bacc("TRN2", target_bir_lowering=False, debug=False,
                       num_devices=N_CORES)
        with tile.TileContext(nc) as tc:
            _body(tc)
        nc.compile()
        _NC_CACHE = nc
    return _NC_CACHE


def _in_maps(x, W_Q, W_K, W_V, W_O):
    from ml_dtypes import bfloat16
    x = np.asarray(x, dtype=np.float32)
    W_Q = np.asarray(W_Q, dtype=np.float32).astype(bfloat16)
    W_K = np.asarray(W_K, dtype=np.float32).astype(bfloat16)
    W_V = np.asarray(W_V, dtype=np.float32).astype(bfloat16)
    W_O = np.asarray(W_O, dtype=np.float32).astype(bfloat16)
    xTs = [np.ascontiguousarray(x[b].T).astype(bfloat16) for b in range(B)]
    maps = []
    for i in range(N_CORES):
        b, k = i // 4, i % 4
        sl = slice(DL * k, DL * k + DL)
        maps.append({
            "xT": xTs[b],
            "wq": np.ascontiguousarray(W_Q[:, sl]),
            "wk": np.ascontiguousarray(W_K[:, sl]),
            "wv": np.ascontiguousarray(W_V[:, sl]),
            "wo": np.ascontiguousarray(W_O[sl, :]),
        })
    return maps


def _gather(results):
    out = np.zeros([B, S, D], np.float32)
    for b in range(B):
        acc = np.zeros([D, S], np.float64)
        for i in range(4 * b, 4 * b + 4):
            acc += np.asarray(results[i]["outT"], np.float32)
        out[b] = acc.T
    return out


def kernel(x, W_Q, W_K, W_V, W_O):
    nc = _get_nc()
    res = run_bass_kernel_spmd(nc, _in_maps(x, W_Q, W_K, W_V, W_O),
                               core_ids=list(range(N_CORES)))
    return _gather(res.results)


LAST_RESULT = None


def kernel_profiled(x, W_Q, W_K, W_V, W_O):
    """Like kernel() but with NTFF tracing; returns (output, exec_time_ns)."""
    import os
    global LAST_RESULT
    nc = _get_nc()
    res = run_bass_kernel_spmd(nc, _in_maps(x, W_Q, W_K, W_V, W_O),
                               core_ids=list(range(N_CORES)), trace=True,
                               tmpdir=os.environ.get("BASS_TRACE_DIR"))
    LAST_RESULT = res
    return _gather(res.results), res.exec_time_ns


# revision 7
# speedup vs baseline: 2.2400x; 1.0262x over previous
"""Causal multi-head attention on 8 Trainium2 NeuronCores — v2.

Problem: x[2,2048,1024] @ W_Q/K/V[1024,1024] -> 16-head causal attention
(d_head=64) -> @ W_O[1024,1024].

Sharding: DP(batch=2) x TP(head-groups=4). Core i owns batch i//4 and
heads 4k..4k+3 where k = i%4 (columns [256k:256k+256) of W_Q/K/V, rows
[256k:256k+256) of W_O). Each core emits a partial [1024, 2048] output
for its batch; host sums groups of 4 and transposes.

All matmul inputs bf16 (f32 PSUM accumulate). Design notes:
  - Attention runs in two "waves" (head pairs), so PSUM fits: scores
    double-buffered [128,2,512] (4 banks) + PV accum [65,2,512]
    (2 banks) + utility [128,512] x2 (2 banks) = 8 banks.
  - exp batched over the head pair: one ACTIVATE per (jj, chunk) covers
    [128, 2, width] -> halves ScalarE fixed overhead.
  - V projected directly in [token, dim] orientation (x-chunk stationary,
    W_V moving) — no PE transposes.
  - Projection work is emitted just-in-time into wave-A chunk loops (Q
    before its q-tile, K/V before their k-chunks); W_O blocks interleave
    into wave-B (jj order 1,2,3,0 so the tail lands on the shortest jj).
    Keeps the PE stream dense so the HAM clock gate stays at 2.4 GHz.
  - W_O results bounce PSUM->SBUF as bf16 (ScalarE/DVE alternating) and
    store with one DMA per token tile; host sums bf16 partials.
  - Softmax denominator via ones-column in vn ([65]-wide PV stationary).
    PV accumulators drain to SBUF incrementally (column quarters finalize
    as the causal diagonal passes), so the PSUM bank handoff to the next
    q-tile never stalls. Normalize = reciprocal_approx_fast on the
    denominator row, gpsimd partition broadcast, DVE multiply — all off
    the PE critical path; the final q-tile normalizes straight from PSUM.
"""

import numpy as np

import concourse.bass as bass
import concourse.tile as tile
from concourse import bacc, mybir
from concourse.bass_utils import run_bass_kernel_spmd

F32 = mybir.dt.float32
BF16 = mybir.dt.bfloat16

N_CORES = 8
P = 128
D = 1024          # d_model
B = 2             # batch
S = 2048          # seq len (= tokens per core)
TT = 512          # token tile
NT = S // TT      # 4 token tiles
KD = D // P       # 8 contraction chunks for projections
CB = S // P       # 16 k-chunks
HL = 4            # heads per core
DL = 256          # dims per core (2 ptiles of 128)
DH = 64           # head dim
EXP = mybir.ActivationFunctionType.Exp


def _body(tc):
    nc = tc.nc
    # all host-side pre-arranged so every DMA is contiguous
    xT = nc.dram_tensor("xT", [P, KD, S], BF16, kind="ExternalInput").ap()
    wq = nc.dram_tensor("wq", [P, KD, DL], BF16, kind="ExternalInput").ap()
    wk = nc.dram_tensor("wk", [P, KD, DL], BF16, kind="ExternalInput").ap()
    wv = nc.dram_tensor("wv", [P, KD, DL], BF16, kind="ExternalInput").ap()
    wo = nc.dram_tensor("wo", [P, 2, D], BF16, kind="ExternalInput").ap()
    outT = nc.dram_tensor("outT", [P, KD, S], BF16, kind="ExternalOutput").ap()

    import contextlib
    with contextlib.ExitStack() as ctx:
        const = ctx.enter_context(tc.tile_pool(name="const", bufs=1))
        wpool = ctx.enter_context(tc.tile_pool(name="wpool", bufs=1))
        xpool = ctx.enter_context(tc.tile_pool(name="xpool", bufs=2))
        persist = ctx.enter_context(tc.tile_pool(name="persist", bufs=1))
        probs_p = ctx.enter_context(tc.tile_pool(name="probs", bufs=4))
        stage = ctx.enter_context(tc.tile_pool(name="stage", bufs=2))
        psum = ctx.enter_context(tc.tile_pool(name="psum", bufs=1, space="PSUM"))

        # --- constants -----------------------------------------------------
        # mask_band[k, q] = 1.0 if q >= k else 0.0
        mask_band = const.tile([P, P], BF16)
        nc.any.memset(mask_band[:], 1.0)
        nc.gpsimd.affine_select(
            out=mask_band[:],
            in_=mask_band[:],
            compare_op=mybir.AluOpType.is_ge,
            fill=0.0,
            base=0,
            pattern=[[1, P]],
            channel_multiplier=-1,
        )
        # preload the exp table set while projections run
        scr = const.tile([1, 1], F32)
        nc.any.memset(scr[:], 0.0)
        nc.scalar.activation(scr[:], scr[:], EXP)

        # --- weights -------------------------------------------------------
        wq_sb = wpool.tile([P, KD, DL], BF16)
        nc.sync.dma_start(wq_sb[:], wq)
        wk_sb = wpool.tile([P, KD, DL], BF16)
        nc.sync.dma_start(wk_sb[:], wk)
        wv_sb = wpool.tile([P, KD, DL], BF16)
        nc.sync.dma_start(wv_sb[:], wv)
        wo_sb = wpool.tile([P, 2, D], BF16)
        nc.sync.dma_start(wo_sb[:], wo)

        # --- persistent activations ---------------------------------------
        qT = persist.tile([P, 2, S], BF16)    # [dim%128, ptile, token]
        kT = persist.tile([P, 2, S], BF16)
        attnT = persist.tile([P, 2, S], BF16)
        vn = persist.tile([P, CB, HL, DH + 1], BF16)  # [tok, chunk, head, d|1]
        nc.any.memset(vn[:, :, :, DH], 1.0)
        pvsb = persist.tile([DH + 1, NT, 2, TT], F32)  # [d|1, jj, hh, q]


        # --- projection work for one token tile, as emission groups -------
        # Returns (load, [Q chains], [K chains], [V chunks]). Q must be
        # emitted before wave-A jj=t; K/V only before chunk 4t of jj=t.
        xts = {t: xpool.tile([P, KD, TT], BF16, name=f"xt_{t}")
               for t in range(NT)}

        def proj_load(t):
            def run():
                # per-chunk DMAs: subtile deps let chain c start as soon as
                # its own chunk lands, instead of waiting for the full tile
                for c in range(KD):
                    nc.sync.dma_start(xts[t][:, c, :],
                                      xT[:, c, bass.ts(t, TT)])
            return run

        def proj_tile_groups(t):
            tsl = bass.ts(t, TT)
            xt = xts[t]

            def qk_chain(wsb, dst, pt):
                def run():
                    ps = psum.tile([P, TT], F32, tag="u", bufs=2,
                                   name=f"ps_{t}_{pt}")
                    for c in range(KD):
                        nc.tensor.matmul(ps[:], wsb[:, c, bass.ts(pt, P)],
                                         xt[:, c, :],
                                         start=(c == 0), stop=(c == KD - 1))
                    nc.vector.tensor_copy(dst[:, pt, tsl], ps[:])
                return run

            # V directly in [token, dim] orientation: x chunk stationary.
            def v_chunk(s_):
                def run():
                    ch = t * 4 + s_
                    ps = psum.tile([P, DL], F32, tag="u", bufs=2,
                                   name=f"psv_{t}_{s_}")
                    for c in range(KD):
                        nc.tensor.matmul(ps[:], xt[:, c, bass.ts(s_, P)],
                                         wv_sb[:, c, :],
                                         start=(c == 0), stop=(c == KD - 1))
                    # [128 tok, 4*64 dims] -> vn[:, ch, h, 0:64]
                    nc.vector.tensor_copy(
                        vn[:, ch, :, 0:DH],
                        ps.rearrange("p (h d) -> p h d", h=HL))
                return run

            qs = [qk_chain(wq_sb, qT, pt) for pt in range(2)]
            ks = [qk_chain(wk_sb, kT, pt) for pt in range(2)]
            vs = [v_chunk(s_) for s_ in range(4)]
            return qs, ks, vs

        # --- W_O for one token tile, as one emission group ----------------
        def wo_group(jx):
            jsl = bass.ts(jx, TT)

            def run():
                ob = stage.tile([P, KD, TT], BF16, tag="ob", bufs=2,
                                name=f"ob_{jx}")
                for f in range(KD):
                    wu = psum.tile([P, TT], F32, tag="u", bufs=2,
                                   name=f"wu_{jx}_{f}")
                    nc.tensor.matmul(wu[:], wo_sb[:, 0, bass.ts(f, P)],
                                     attnT[:, 0, jsl], start=True, stop=False)
                    nc.tensor.matmul(wu[:], wo_sb[:, 1, bass.ts(f, P)],
                                     attnT[:, 1, jsl], start=False, stop=True)
                    if f % 2 == 0:
                        nc.scalar.copy(ob[:, f, :], wu[:])
                    else:
                        nc.vector.tensor_copy(ob[:, f, :], wu[:])
                nc.sync.dma_start(outT[:, :, jsl], ob[:])
            return run

        # --- attention wave: one head pair (ptile), all q-tiles -----------
        def norm_jj(pt, jj):
            jsl = bass.ts(jj, TT)
            for hh in range(2):
                # reciprocal_approx_fast misreads inputs at base partition
                # 64 (custom-DVE uop quirk) — bounce the row to partition 0.
                dcp = stage.tile([1, TT], F32, tag="dcp",
                                 name=f"dcp_{pt}_{jj}_{hh}")
                nc.vector.tensor_copy(dcp[:], pvsb[DH:DH + 1, jj, hh, :])
                rc = stage.tile([1, TT], F32, tag="rc",
                                name=f"rc_{pt}_{jj}_{hh}")
                nc.vector.reciprocal_approx_fast(out=rc[:], in_=dcp[:])
                rb = stage.tile([DH, TT], F32, tag="rb",
                                name=f"rb_{pt}_{jj}_{hh}")
                nc.gpsimd.partition_broadcast(rb[:], rc[:])
                nc.vector.tensor_mul(
                    attnT[DH * hh:DH * hh + DH, pt, jsl],
                    pvsb[0:DH, jj, hh, :], rb[:])

        def wave(pt, extra_per_jj, order=None, fast_last=False):
            """extra_per_jj[i]: emission groups interleaved into the i-th
            processed jj's chunk loop (all emitted before its normalize)."""
            order = list(order) if order else list(range(NT))
            for idx, jj in enumerate(order):
                extra = list(extra_per_jj[idx])
                ncb = 4 * (jj + 1)
                jsl = bass.ts(jj, TT)
                pv = psum.tile([DH + 1, 2, TT], F32, tag="pv", bufs=1,
                               name=f"pv_{pt}_{jj}")

                def pv_step(cb, pr, jj=jj, ncb=ncb, pv=pv):
                    r = cb - 4 * jj
                    lo = P * r if r > 0 else 0
                    for hh in range(2):
                        nc.tensor.matmul(pv[:, hh, lo:],
                                         vn[:, cb, 2 * pt + hh, :],
                                         pr[:, hh, lo:],
                                         start=(cb == 0), stop=(cb == ncb - 1))

                pending = None
                for cb in range(ncb):
                    r = cb - 4 * jj
                    lo = P * r if r > 0 else 0
                    csl = bass.ts(cb, P)
                    sps = psum.tile([P, 2, TT], F32, tag="s", bufs=2,
                                    name=f"sps_{pt}_{jj}_{cb}")
                    for hh in range(2):
                        hp = slice(DH * hh, DH * hh + DH)
                        nc.tensor.matmul(sps[:, hh, lo:], kT[hp, pt, csl],
                                         qT[hp, pt, jsl][:, lo:],
                                         start=True, stop=True)
                    pr = probs_p.tile([P, 2, TT], BF16, tag="pr",
                                      name=f"pr_{pt}_{jj}_{cb}")
                    nc.scalar.activation(pr[:, :, lo:], sps[:, :, lo:],
                                         EXP, scale=0.125)
                    if r >= 0:
                        for hh in range(2):
                            nc.vector.tensor_mul(pr[:, hh, bass.ts(r, P)],
                                                 pr[:, hh, bass.ts(r, P)],
                                                 mask_band[:])
                    if pending is not None:
                        pv_step(cb - 1, pending)
                        rq = cb - 1 - 4 * jj
                        if rq >= 0 and not (fast_last and idx == NT - 1):
                            # columns [128rq,128rq+128) of pv are final now:
                            # drain incrementally so the bank frees right
                            # after the last PV instead of one big copy late
                            qsl = bass.ts(rq, P)
                            if pt == 0:
                                nc.scalar.copy(pvsb[:, jj, :, qsl],
                                               pv[:, :, qsl])
                            else:
                                nc.vector.tensor_copy(pvsb[:, jj, :, qsl],
                                                      pv[:, :, qsl])
                    pending = pr
                    # spread the extra groups across remaining chunk slots
                    k = -(-len(extra) // (ncb - cb)) if extra else 0
                    for _ in range(k):
                        extra.pop(0)()
                pv_step(ncb - 1, pending)
                if not (fast_last and idx == NT - 1):
                    qsl = bass.ts(3, P)
                    if pt == 0:
                        nc.scalar.copy(pvsb[:, jj, :, qsl], pv[:, :, qsl])
                    else:
                        nc.vector.tensor_copy(pvsb[:, jj, :, qsl],
                                              pv[:, :, qsl])
                while extra:
                    extra.pop(0)()

                if fast_last and idx == NT - 1:
                    # final jj of the kernel: nothing needs the PSUM banks
                    # next, so normalize straight out of PSUM (shortest
                    # latency into the last W_O block)
                    dcp2 = stage.tile([1, 2, TT], F32, tag="dcp2",
                                      name=f"dcp2_{pt}_{jj}")
                    nc.vector.tensor_copy(dcp2[:], pv[DH:DH + 1, :, :])
                    rc2 = stage.tile([1, 2, TT], F32, tag="rc2",
                                     name=f"rc2_{pt}_{jj}")
                    nc.vector.reciprocal_approx_fast(out=rc2[:], in_=dcp2[:])
                    for hh in range(2):
                        rb = stage.tile([DH, TT], F32, tag="rb",
                                        name=f"rbl_{pt}_{jj}_{hh}")
                        nc.gpsimd.partition_broadcast(rb[:], rc2[:, hh, :])
                        nc.vector.tensor_mul(
                            attnT[DH * hh:DH * hh + DH, pt, jsl],
                            pv[0:DH, hh, :], rb[:])
                    continue
                norm_jj(pt, jj)

        # --- schedule ------------------------------------------------------
        proj_load(0)()
        proj_load(1)()
        pg = {t: proj_tile_groups(t) for t in range(NT)}
        for g in pg[0][0] + pg[0][1] + pg[0][2]:   # tile 0: Q, K, V
            g()
        wave(0, [
            pg[1][0] + [proj_load(2)],                      # jj0: Q1, L2
            pg[1][1] + pg[1][2] + pg[2][0] + [proj_load(3)],  # K1 V1 Q2 L3
            pg[2][1] + pg[2][2] + pg[3][0],                 # K2 V2 Q3
            pg[3][1] + pg[3][2],                            # K3 V3
        ])
        wave(1, [[], [wo_group(1)], [wo_group(2)], [wo_group(3)]],
             order=[1, 2, 3, 0], fast_last=True)
        wo_group(0)()


_NC_CACHE = None


def _get_nc():
    global _NC_CACHE
    if _NC_CACHE is None:
        nc = bacc.Bacc("TRN2", target_bir_lowering=False, debug=False,
                       num_devices=N_CORES)
        with tile.TileContext(nc) as tc:
            _body(tc)
        nc.compile()
        _NC_CACHE = nc
    return _NC_CACHE


def _pom(w):
    """[o*P+p, m] -> [p, o, m] (contiguous)."""
    o = w.shape[0] // P
    return np.ascontiguousarray(w.reshape(o, P, -1).transpose(1, 0, 2))


def _in_maps(x, W_Q, W_K, W_V, W_O):
    from ml_dtypes import bfloat16
    x = np.asarray(x, dtype=np.float32)
    W_Q = np.asarray(W_Q, dtype=np.float32).astype(bfloat16)
    W_K = np.asarray(W_K, dtype=np.float32).astype(bfloat16)
    W_V = np.asarray(W_V, dtype=np.float32).astype(bfloat16)
    W_O = np.asarray(W_O, dtype=np.float32).astype(bfloat16)
    xTs = [_pom(np.ascontiguousarray(x[b].T).astype(bfloat16))
           for b in range(B)]
    maps = []
    for i in range(N_CORES):
        b, k = i // 4, i % 4
        sl = slice(DL * k, DL * k + DL)
        maps.append({
            "xT": xTs[b],
            "wq": _pom(W_Q[:, sl]),
            "wk": _pom(W_K[:, sl]),
            "wv": _pom(W_V[:, sl]),
            "wo": _pom(W_O[sl, :]),
        })
    return maps


def _gather(results):
    out = np.zeros([B, S, D], np.float32)
    for b in range(B):
        acc = np.zeros([D, S], np.float64)
        for i in range(4 * b, 4 * b + 4):
            # [p, o, n] -> [o*P+p, n]
            acc += np.asarray(results[i]["outT"],
                              np.float32).transpose(1, 0, 2).reshape(D, S)
        out[b] = acc.T
    return out


def kernel(x, W_Q, W_K, W_V, W_O):
    nc = _get_nc()
    res = run_bass_kernel_spmd(nc, _in_maps(x, W_Q, W_K, W_V, W_O),
                               core_ids=list(range(N_CORES)))
    return _gather(res.results)


LAST_RESULT = None


def kernel_profiled(x, W_Q, W_K, W_V, W_O):
    """Like kernel() but with NTFF tracing; returns (output, exec_time_ns)."""
    import os
    global LAST_RESULT
    nc = _get_nc()
    res = run_bass_kernel_spmd(nc, _in_maps(x, W_Q, W_K, W_V, W_O),
                               core_ids=list(range(N_CORES)), trace=True,
                               tmpdir=os.environ.get("BASS_TRACE_DIR"))
    LAST_RESULT = res
    return _gather(res.results), res.exec_time_ns


# revision 8
# speedup vs baseline: 2.2915x; 1.0230x over previous
"""Causal multi-head attention on 8 Trainium2 NeuronCores — v2.

Problem: x[2,2048,1024] @ W_Q/K/V[1024,1024] -> 16-head causal attention
(d_head=64) -> @ W_O[1024,1024].

Sharding: DP(batch=2) x TP(head-groups=4). Core i owns batch i//4 and
heads 4k..4k+3 where k = i%4 (columns [256k:256k+256) of W_Q/K/V, rows
[256k:256k+256) of W_O). Each core emits a partial [1024, 2048] output
for its batch; host sums groups of 4 and transposes.

All matmul inputs bf16 (f32 PSUM accumulate). Design notes:
  - Attention runs in two "waves" (head pairs), so PSUM fits: scores
    double-buffered [128,2,512] (4 banks) + PV accum [65,2,512]
    (2 banks) + utility [128,512] x2 (2 banks) = 8 banks.
  - exp batched over the head pair: one ACTIVATE per (jj, chunk) covers
    [128, 2, width] -> halves ScalarE fixed overhead.
  - V projected directly in [token, dim] orientation (x-chunk stationary,
    W_V moving) — no PE transposes.
  - Projection work is emitted just-in-time into wave-A chunk loops (Q
    before its q-tile, K/V before their k-chunks); W_O blocks interleave
    into wave-B (jj order 1,2,3,0 so the tail lands on the shortest jj).
    Keeps the PE stream dense so the HAM clock gate stays at 2.4 GHz.
  - W_O results bounce PSUM->SBUF as bf16 (ScalarE/DVE alternating) and
    store with one DMA per token tile; host sums bf16 partials.
  - Softmax denominator via ones-column in vn ([65]-wide PV stationary).
    PV accumulators drain to SBUF incrementally (column quarters finalize
    as the causal diagonal passes), so the PSUM bank handoff to the next
    q-tile never stalls. Normalize = reciprocal_approx_fast on the
    denominator row, gpsimd partition broadcast, DVE multiply — all off
    the PE critical path; the final q-tile normalizes straight from PSUM.
"""

import numpy as np

import concourse.bass as bass
import concourse.tile as tile
from concourse import bacc, mybir
from concourse.bass_utils import run_bass_kernel_spmd

F32 = mybir.dt.float32
BF16 = mybir.dt.bfloat16

N_CORES = 8
P = 128
D = 1024          # d_model
B = 2             # batch
S = 2048          # seq len (= tokens per core)
TT = 512          # token tile
NT = S // TT      # 4 token tiles
KD = D // P       # 8 contraction chunks for projections
CB = S // P       # 16 k-chunks
HL = 4            # heads per core
DL = 256          # dims per core (2 ptiles of 128)
DH = 64           # head dim
EXP = mybir.ActivationFunctionType.Exp


def _body(tc):
    nc = tc.nc
    # all host-side pre-arranged so every DMA is contiguous
    xT = nc.dram_tensor("xT", [P, KD, S], BF16, kind="ExternalInput").ap()
    wq = nc.dram_tensor("wq", [P, KD, DL], BF16, kind="ExternalInput").ap()
    wk = nc.dram_tensor("wk", [P, KD, DL], BF16, kind="ExternalInput").ap()
    wv = nc.dram_tensor("wv", [P, KD, DL], BF16, kind="ExternalInput").ap()
    wo = nc.dram_tensor("wo", [P, 2, D], BF16, kind="ExternalInput").ap()
    outT = nc.dram_tensor("outT", [P, KD, S], BF16, kind="ExternalOutput").ap()

    import contextlib
    with contextlib.ExitStack() as ctx:
        const = ctx.enter_context(tc.tile_pool(name="const", bufs=1))
        wpool = ctx.enter_context(tc.tile_pool(name="wpool", bufs=1))
        xpool = ctx.enter_context(tc.tile_pool(name="xpool", bufs=2))
        persist = ctx.enter_context(tc.tile_pool(name="persist", bufs=1))
        probs_p = ctx.enter_context(tc.tile_pool(name="probs", bufs=4))
        stage = ctx.enter_context(tc.tile_pool(name="stage", bufs=2))
        psum = ctx.enter_context(tc.tile_pool(name="psum", bufs=1, space="PSUM"))

        # --- constants -----------------------------------------------------
        # mask_band[k, q] = 1.0 if q >= k else 0.0
        mask_band = const.tile([P, P], BF16)
        nc.any.memset(mask_band[:], 1.0)
        nc.gpsimd.affine_select(
            out=mask_band[:],
            in_=mask_band[:],
            compare_op=mybir.AluOpType.is_ge,
            fill=0.0,
            base=0,
            pattern=[[1, P]],
            channel_multiplier=-1,
        )
        # preload the exp table set while projections run
        scr = const.tile([1, 1], F32)
        nc.any.memset(scr[:], 0.0)
        nc.scalar.activation(scr[:], scr[:], EXP)

        # --- weights -------------------------------------------------------
        wq_sb = wpool.tile([P, KD, DL], BF16)
        wk_sb = wpool.tile([P, KD, DL], BF16)
        wv_sb = wpool.tile([P, KD, DL], BF16)
        wo_sb = wpool.tile([P, 2, D], BF16)

        # --- persistent activations ---------------------------------------
        qT = persist.tile([P, 2, S], BF16)    # [dim%128, ptile, token]
        kT = persist.tile([P, 2, S], BF16)
        attnT = persist.tile([P, 2, S], BF16)
        vn = persist.tile([P, CB, HL, DH + 1], BF16)  # [tok, chunk, head, d|1]
        nc.any.memset(vn[:, :, :, DH], 1.0)
        pvsb = persist.tile([DH + 1, NT, 2, TT], F32)  # [d|1, jj, hh, q]


        # --- projection work for one token tile, as emission groups -------
        # Returns (load, [Q chains], [K chains], [V chunks]). Q must be
        # emitted before wave-A jj=t; K/V only before chunk 4t of jj=t.
        xts = {t: xpool.tile([P, KD, TT], BF16, name=f"xt_{t}")
               for t in range(NT)}

        def proj_load(t):
            def run():
                # per-chunk DMAs: subtile deps let chain c start as soon as
                # its own chunk lands, instead of waiting for the full tile
                for c in range(KD):
                    nc.sync.dma_start(xts[t][:, c, :],
                                      xT[:, c, bass.ts(t, TT)])
            return run

        def proj_tile_groups(t):
            tsl = bass.ts(t, TT)
            xt = xts[t]

            def qk_chain(wsb, dst, pt):
                def run():
                    ps = psum.tile([P, TT], F32, tag="u", bufs=2,
                                   name=f"ps_{t}_{pt}")
                    for c in range(KD):
                        nc.tensor.matmul(ps[:], wsb[:, c, bass.ts(pt, P)],
                                         xt[:, c, :],
                                         start=(c == 0), stop=(c == KD - 1))
                    nc.vector.tensor_copy(dst[:, pt, tsl], ps[:])
                return run

            # V directly in [token, dim] orientation: x chunk stationary.
            def v_chunk(s_):
                def run():
                    ch = t * 4 + s_
                    ps = psum.tile([P, DL], F32, tag="u", bufs=2,
                                   name=f"psv_{t}_{s_}")
                    for c in range(KD):
                        nc.tensor.matmul(ps[:], xt[:, c, bass.ts(s_, P)],
                                         wv_sb[:, c, :],
                                         start=(c == 0), stop=(c == KD - 1))
                    # [128 tok, 4*64 dims] -> vn[:, ch, h, 0:64]
                    nc.vector.tensor_copy(
                        vn[:, ch, :, 0:DH],
                        ps.rearrange("p (h d) -> p h d", h=HL))
                return run

            qs = [qk_chain(wq_sb, qT, pt) for pt in range(2)]
            ks = [qk_chain(wk_sb, kT, pt) for pt in range(2)]
            vs = [v_chunk(s_) for s_ in range(4)]
            return qs, ks, vs

        # --- W_O for one token tile, as one emission group ----------------
        def wo_group(jx):
            jsl = bass.ts(jx, TT)

            def run():
                ob = stage.tile([P, KD, TT], BF16, tag="ob", bufs=2,
                                name=f"ob_{jx}")
                for f in range(KD):
                    wu = psum.tile([P, TT], F32, tag="u", bufs=2,
                                   name=f"wu_{jx}_{f}")
                    nc.tensor.matmul(wu[:], wo_sb[:, 0, bass.ts(f, P)],
                                     attnT[:, 0, jsl], start=True, stop=False)
                    nc.tensor.matmul(wu[:], wo_sb[:, 1, bass.ts(f, P)],
                                     attnT[:, 1, jsl], start=False, stop=True)
                    if f % 2 == 0:
                        nc.scalar.copy(ob[:, f, :], wu[:])
                    else:
                        nc.vector.tensor_copy(ob[:, f, :], wu[:])
                    nc.sync.dma_start(outT[:, f, jsl], ob[:, f, :])
            return run

        # --- attention wave: one head pair (ptile), all q-tiles -----------
        def norm_jj(pt, jj):
            jsl = bass.ts(jj, TT)
            for hh in range(2):
                # reciprocal_approx_fast misreads inputs at base partition
                # 64 (custom-DVE uop quirk) — bounce the row to partition 0.
                dcp = stage.tile([1, TT], F32, tag="dcp",
                                 name=f"dcp_{pt}_{jj}_{hh}")
                nc.vector.tensor_copy(dcp[:], pvsb[DH:DH + 1, jj, hh, :])
                rc = stage.tile([1, TT], F32, tag="rc",
                                name=f"rc_{pt}_{jj}_{hh}")
                nc.vector.reciprocal_approx_fast(out=rc[:], in_=dcp[:])
                rb = stage.tile([DH, TT], F32, tag="rb",
                                name=f"rb_{pt}_{jj}_{hh}")
                nc.gpsimd.partition_broadcast(rb[:], rc[:])
                nc.vector.tensor_mul(
                    attnT[DH * hh:DH * hh + DH, pt, jsl],
                    pvsb[0:DH, jj, hh, :], rb[:])

        def wave(pt, extra_per_jj, order=None, fast_last=False):
            """extra_per_jj[i]: emission groups interleaved into the i-th
            processed jj's chunk loop (all emitted before its normalize)."""
            order = list(order) if order else list(range(NT))
            for idx, jj in enumerate(order):
                extra = list(extra_per_jj[idx])
                ncb = 4 * (jj + 1)
                jsl = bass.ts(jj, TT)
                pv = psum.tile([DH + 1, 2, TT], F32, tag="pv", bufs=1,
                               name=f"pv_{pt}_{jj}")

                def pv_step(cb, pr, jj=jj, ncb=ncb, pv=pv):
                    r = cb - 4 * jj
                    lo = P * r if r > 0 else 0
                    for hh in range(2):
                        nc.tensor.matmul(pv[:, hh, lo:],
                                         vn[:, cb, 2 * pt + hh, :],
                                         pr[:, hh, lo:],
                                         start=(cb == 0), stop=(cb == ncb - 1))

                pending = None
                for cb in range(ncb):
                    r = cb - 4 * jj
                    lo = P * r if r > 0 else 0
                    csl = bass.ts(cb, P)
                    sps = psum.tile([P, 2, TT], F32, tag="s", bufs=2,
                                    name=f"sps_{pt}_{jj}_{cb}")
                    for hh in range(2):
                        hp = slice(DH * hh, DH * hh + DH)
                        nc.tensor.matmul(sps[:, hh, lo:], kT[hp, pt, csl],
                                         qT[hp, pt, jsl][:, lo:],
                                         start=True, stop=True)
                    pr = probs_p.tile([P, 2, TT], BF16, tag="pr",
                                      name=f"pr_{pt}_{jj}_{cb}")
                    nc.scalar.activation(pr[:, :, lo:], sps[:, :, lo:],
                                         EXP, scale=0.125)
                    if r >= 0:
                        for hh in range(2):
                            nc.vector.tensor_mul(pr[:, hh, bass.ts(r, P)],
                                                 pr[:, hh, bass.ts(r, P)],
                                                 mask_band[:])
                    if pending is not None:
                        pv_step(cb - 1, pending)
                        rq = cb - 1 - 4 * jj
                        if rq >= 0 and not (fast_last and idx == NT - 1):
                            # columns [128rq,128rq+128) of pv are final now:
                            # drain incrementally so the bank frees right
                            # after the last PV instead of one big copy late
                            qsl = bass.ts(rq, P)
                            if pt == 0:
                                nc.scalar.copy(pvsb[:, jj, :, qsl],
                                               pv[:, :, qsl])
                            else:
                                nc.vector.tensor_copy(pvsb[:, jj, :, qsl],
                                                      pv[:, :, qsl])
                    pending = pr
                    # spread the extra groups across remaining chunk slots
                    k = -(-len(extra) // (ncb - cb)) if extra else 0
                    for _ in range(k):
                        extra.pop(0)()
                pv_step(ncb - 1, pending)
                if not (fast_last and idx == NT - 1):
                    qsl = bass.ts(3, P)
                    if pt == 0:
                        nc.scalar.copy(pvsb[:, jj, :, qsl], pv[:, :, qsl])
                    else:
                        nc.vector.tensor_copy(pvsb[:, jj, :, qsl],
                                              pv[:, :, qsl])
                while extra:
                    extra.pop(0)()

                if fast_last and idx == NT - 1:
                    # final jj of the kernel: nothing needs the PSUM banks
                    # next, so normalize straight out of PSUM (shortest
                    # latency into the last W_O block)
                    dcp2 = stage.tile([1, 2, TT], F32, tag="dcp2",
                                      name=f"dcp2_{pt}_{jj}")
                    nc.vector.tensor_copy(dcp2[:], pv[DH:DH + 1, :, :])
                    rc2 = stage.tile([1, 2, TT], F32, tag="rc2",
                                     name=f"rc2_{pt}_{jj}")
                    nc.vector.reciprocal_approx_fast(out=rc2[:], in_=dcp2[:])
                    for hh in range(2):
                        rb = stage.tile([DH, TT], F32, tag="rb",
                                        name=f"rbl_{pt}_{jj}_{hh}")
                        nc.gpsimd.partition_broadcast(rb[:], rc2[:, hh, :])
                        nc.vector.tensor_mul(
                            attnT[DH * hh:DH * hh + DH, pt, jsl],
                            pv[0:DH, hh, :], rb[:])
                    continue
                norm_jj(pt, jj)

        # --- schedule ------------------------------------------------------
        # DMA issue order = packet priority on the shared queues: the first
        # Q chain needs wq + xt0 only; wo isn't read until wave B.
        nc.sync.dma_start(wq_sb[:], wq)
        proj_load(0)()
        nc.sync.dma_start(wk_sb[:], wk)
        nc.sync.dma_start(wv_sb[:], wv)
        proj_load(1)()
        nc.sync.dma_start(wo_sb[:], wo)
        pg = {t: proj_tile_groups(t) for t in range(NT)}
        for g in pg[0][0] + pg[0][1] + pg[0][2]:   # tile 0: Q, K, V
            g()
        wave(0, [
            pg[1][0] + [proj_load(2)],                      # jj0: Q1, L2
            pg[1][1] + pg[1][2] + pg[2][0] + [proj_load(3)],  # K1 V1 Q2 L3
            pg[2][1] + pg[2][2] + pg[3][0],                 # K2 V2 Q3
            pg[3][1] + pg[3][2],                            # K3 V3
        ])
        wave(1, [[], [wo_group(1)], [wo_group(2)], [wo_group(3)]],
             order=[1, 2, 3, 0], fast_last=True)
        wo_group(0)()


_NC_CACHE = None


def _get_nc():
    global _NC_CACHE
    if _NC_CACHE is None:
        nc = bacc.Bacc("TRN2", target_bir_lowering=False, debug=False,
                       num_devices=N_CORES)
        with tile.TileContext(nc) as tc:
            _body(tc)
        nc.compile()
        _NC_CACHE = nc
    return _NC_CACHE


def _pom(w):
    """[o*P+p, m] -> [p, o, m] (contiguous)."""
    o = w.shape[0] // P
    return np.ascontiguousarray(w.reshape(o, P, -1).transpose(1, 0, 2))


def _in_maps(x, W_Q, W_K, W_V, W_O):
    from ml_dtypes import bfloat16
    x = np.asarray(x, dtype=np.float32)
    W_Q = np.asarray(W_Q, dtype=np.float32).astype(bfloat16)
    W_K = np.asarray(W_K, dtype=np.float32).astype(bfloat16)
    W_V = np.asarray(W_V, dtype=np.float32).astype(bfloat16)
    W_O = np.asarray(W_O, dtype=np.float32).astype(bfloat16)
    xTs = [_pom(np.ascontiguousarray(x[b].T).astype(bfloat16))
           for b in range(B)]
    maps = []
    for i in range(N_CORES):
        b, k = i // 4, i % 4
        sl = slice(DL * k, DL * k + DL)
        maps.append({
            "xT": xTs[b],
            "wq": _pom(W_Q[:, sl]),
            "wk": _pom(W_K[:, sl]),
            "wv": _pom(W_V[:, sl]),
            "wo": _pom(W_O[sl, :]),
        })
    return maps


def _gather(results):
    out = np.zeros([B, S, D], np.float32)
    for b in range(B):
        acc = np.zeros([D, S], np.float64)
        for i in range(4 * b, 4 * b + 4):
            # [p, o, n] -> [o*P+p, n]
            acc += np.asarray(results[i]["outT"],
                              np.float32).transpose(1, 0, 2).reshape(D, S)
        out[b] = acc.T
    return out


def kernel(x, W_Q, W_K, W_V, W_O):
    nc = _get_nc()
    res = run_bass_kernel_spmd(nc, _in_maps(x, W_Q, W_K, W_V, W_O),
                               core_ids=list(range(N_CORES)))
    return _gather(res.results)


LAST_RESULT = None


def kernel_profiled(x, W_Q, W_K, W_V, W_O):
    """Like kernel() but with NTFF tracing; returns (output, exec_time_ns)."""
    import os
    global LAST_RESULT
    nc = _get_nc()
    res = run_bass_kernel_spmd(nc, _in_maps(x, W_Q, W_K, W_V, W_O),
                               core_ids=list(range(N_CORES)), trace=True,
                               tmpdir=os.environ.get("BASS_TRACE_DIR"))
    LAST_RESULT = res
    return _gather(res.results), res.exec_time_ns
